# revision 37
# baseline (speedup 1.0000x reference)
"""DCT non-local attention (nn_DCTNLAttention11) Trainium2 kernel.

Data-parallel over batch B=8 across 8 NeuronCores; each core processes one
batch element [C=512, HW=16384].  All constants derived from the DCT basis P
are precomputed on host; the per-core device program is:

  1. xT [HW, C] bf16 is DMAed ONCE into a resident SBUF tile
     [128p, 128h, 512c] (n = h*128+p); xPT = P^T @ x^T accumulates 128
     matmuls against the resident chunks as the DMA slices land.
  2. xP (PE transposes), then W-projections off xP (tiny matmuls):
     WqxP^T/WkxP^T/WvxP^T (q/k column-interleaved), WqxP/WkxP, fatt.
  3. Per-n norms: QT/KT chunks [128,128] (q/k interleaved columns) via
     PT-chunk-stationary matmuls; ONE bn_stats per chunk reads PSUM and its
     even/odd stats give sum(q^2) and sum(k^2); batched column math.
  4. Pk = P * (1/lamdk) (ACT, bf16); A_ext = Pk^T @ [P|1] accumulated, bf16.
  5. M1T/rowv = fatt^T @ [A|s]; lamdv_pre columns (batched [128,16] psums);
     lamdq and irqv = lamdq*HW + lamdv_pre (= 1/(rq*rv)) flattened to rows
     64/65 of the PT tensor via a DRAM bounce.
  6. Per 512-col group: U_ext [65,512] = [M1;S;e]^T @ [PT; lamdq; irqv]
     (rows 0-63 = M1^T PT + S (x) lamdq, row 64 = irqv); ACT-drain to bf16
     t (row 64 scales the bias so the rqv drain scale cancels there).
     Per 128-n chunk: psum [128n, 512c] = t_chunk^T @ [gamma*WvxP^T;
     gamma*bias]; fused drain out = psum*rqv[n] + xT_chunk (bf16), split
     Vector/GpSimd; out written bf16 in [HW, C]; host transposes + casts.
"""

import numpy as np
import ml_dtypes
from contextlib import ExitStack

import concourse.bass as bass
import concourse.bacc as bacc
import concourse.tile as tile
from concourse import mybir
from concourse.bass_utils import run_bass_kernel_spmd

F32 = mybir.dt.float32
BF16 = mybir.dt.bfloat16
AF = mybir.ActivationFunctionType
ALU = mybir.AluOpType
BF16_NP = ml_dtypes.bfloat16

B, C, H, W = 8, 512, 128, 128
HW = H * W          # 16384
K = 64              # kept DCT coefficients (8x8 band)
NCH = HW // 128     # 128 n-chunks of 128
NCI = HW // 512     # 32 n-chunks of 512
CCH = C // 128      # 4 c-chunks


def _getP():
    """DCT projection matrix P [HW, K], faithful to the reference."""
    Hs, Ws = H, W
    k = (0, 8, 0, 8)
    ind_h = 2.0 * np.arange(Hs) + 1.0
    Dht = np.stack(
        [np.sqrt(2.0) / np.sqrt(Hs) * np.cos(u * ind_h * np.pi / (2.0 * Hs)) for u in range(Hs)]
    ).astype(np.float32)
    Dht[0, :] = 1.0 / np.sqrt(Hs)
    Dh = Dht.T[:, k[0]:k[1]]
    ind_w = 2.0 * np.arange(Ws) + 1.0
    Dvt = np.stack(
        [np.sqrt(2.0) / np.sqrt(Hs) * np.cos(u * ind_w * np.pi / (2.0 * Ws)) for u in range(Ws)]
    ).astype(np.float32)
    Dvt[0, :] = 1.0 / np.sqrt(Ws)
    Dv = Dvt.T[:, k[2]:k[3]]
    P = np.einsum("hu,wv->hwuv", Dh, Dv).reshape(Hs * Ws, (k[1] - k[0]) * (k[3] - k[2]))
    return np.ascontiguousarray(P.astype(np.float32))


def _build():
    nc = bacc.Bacc("TRN2", target_bir_lowering=False, debug=False, enable_asserts=False)

    xtr = nc.dram_tensor("xtr", [128, NCH, C], BF16, kind="ExternalInput")
    pextb = nc.dram_tensor("pextb", [128, NCH, K + 1], BF16, kind="ExternalInput")
    ptbf = nc.dram_tensor("ptbf", [K, HW], BF16, kind="ExternalInput")
    wcat = nc.dram_tensor("wcat", [128, CCH, 640], BF16, kind="ExternalInput")
    ident = nc.dram_tensor("ident", [128, 128], F32, kind="ExternalInput")
    biasg = nc.dram_tensor("biasg", [1, C], F32, kind="ExternalInput")
    gam = nc.dram_tensor("gam", [1, 1], F32, kind="ExternalInput")
    srecol = nc.dram_tensor("srecol", [K + 1, 2], BF16, kind="ExternalInput")
    out = nc.dram_tensor("out", [128, NCH, C], BF16, kind="ExternalOutput")
    flb = nc.dram_tensor("flbounce", [2, HW], BF16, kind="Internal")

    with tile.TileContext(nc) as tc, ExitStack() as top:
        consts = top.enter_context(tc.tile_pool(name="consts", bufs=1))

        # resident x^T: n = h*128 + p on (p, h); 512 c contiguous
        xt_sb = consts.tile([128, NCH, C], BF16)
        # [PT ; lamdq-row ; irqv-row]
        ptx_sb = consts.tile([K + 2, HW], BF16)
        ident_sb = consts.tile([128, 128], F32)
        bias_sb = consts.tile([1, C], F32)
        gamma_sb = consts.tile([128, 1], F32)
        xpt_sb = consts.tile([K, C], F32)            # xP^T
        xp_sb = consts.tile([128, CCH, K], BF16)     # xP chunks (c on partitions)
        qk_cat = consts.tile([K, 64, 2], BF16)       # q/k interleaved columns
        wqxp_sb = consts.tile([K, K], F32)
        wkxp_sb = consts.tile([K, K], F32)
        fatt_sb = consts.tile([K, K], F32)
        a_s_sb = consts.tile([K, K + 1], F32)        # [A | s]
        m1sT_bf = consts.tile([K + 1, K + 2], BF16)  # [M1 | srow-col | e-col]
        rowv_bf = consts.tile([K, 1], BF16)
        wvg_bf = consts.tile([K + 1, C], BF16)       # [gamma*WvxP^T ; gamma*bias]
        wvg2_bf = consts.tile([K + 2, C], BF16)      # [m1s_ext @ wvg]
        stats = consts.tile([128, NCH, 6], F32)      # bn_stats per chunk
        tmpc = consts.tile([128, NCH], F32)
        lamdq_cols = consts.tile([128, NCH], F32)
        rq_cols = consts.tile([128, NCH], F32)
        rlk_cols = consts.tile([128, NCH], F32)
        lpre_cols = consts.tile([128, NCH], F32)
        rqv_cols = consts.tile([128, NCH], F32)
        irqv_cols = consts.tile([128, NCH], F32)

        # ---- stage 1: xPT = P^T @ x^T  ------------------------------------
        # P chunks (from pextb) land first on the scalar ring; x slices
        # stream on sync+gpsimd rings into the resident tile; consts follow.
        with tc.tile_pool(name="pextp", bufs=1) as pextp:
            pextb_sb = pextp.tile([128, NCH, K + 1], BF16)
            with tc.tile_pool(name="s1psum", bufs=1, space="PSUM") as s1p:
                nc.scalar.dma_start(out=pextb_sb[:, 0:16, :], in_=pextb[:, 0:16, :])
                nc.scalar.dma_start(out=pextb_sb[:, 16:NCH, :], in_=pextb[:, 16:NCH, :])
                # x slices: small first for a fast pipeline start, then 4-chunk
                # blocks (4KB contiguous per partition) over four trigger rings
                bounds = [0, 2, 4]
                while bounds[-1] < NCH:
                    bounds.append(min(bounds[-1] + 4, NCH))
                rings = [nc.sync, nc.gpsimd, nc.scalar]
                for s in range(len(bounds) - 1):
                    lo, hi = bounds[s], bounds[s + 1]
                    eng = rings[s % 3]
                    eng.dma_start(out=xt_sb[:, lo:hi, :], in_=xtr[:, lo:hi, :])

                ps_xpt = s1p.tile([K, C], F32)
                for h in range(NCH):
                    nc.tensor.matmul(
                        ps_xpt, lhsT=pextb_sb[:, h, 0:K], rhs=xt_sb[:, h, :],
                        start=(h == 0), stop=(h == NCH - 1),
                    )
                # defer const loads behind the stage-1 stream
                nc.scalar.dma_start(out=ptx_sb[0:K, :], in_=ptbf[:, :])
                nc.scalar.dma_start(out=ident_sb, in_=ident[:, :])
                nc.scalar.dma_start(out=bias_sb, in_=biasg[:, :])
                nc.vector.memset(m1sT_bf, 0.0)
                nc.scalar.dma_start(out=m1sT_bf[:, K:K + 2], in_=srecol[:, :])
                nc.gpsimd.dma_start(out=gamma_sb, in_=gam[:, :].to_broadcast((128, 1)))
                nc.scalar.activation(out=xpt_sb, in_=ps_xpt, func=AF.Copy)

            # ---- stage 2+3: xP via PE transpose; W projections ------------
            with tc.tile_pool(name="wcatp", bufs=1) as wcatp, \
                 tc.tile_pool(name="s2psum", bufs=2, space="PSUM") as s2p, \
                 tc.tile_pool(name="s3psum", bufs=1, space="PSUM") as s3p:
                wcat_sb = wcatp.tile([128, CCH, 640], BF16)
                nc.scalar.dma_start(out=wcat_sb, in_=wcat[:, :, :])
                for cc in range(CCH):
                    ps_tr = s2p.tile([128, K], F32, tag="tr")
                    nc.tensor.transpose(
                        ps_tr, xpt_sb[:, cc * 128:(cc + 1) * 128], ident_sb[0:K, 0:K]
                    )
                    nc.scalar.activation(out=xp_sb[:, cc, :], in_=ps_tr, func=AF.Copy)

                ps_w1 = s3p.tile([K, 512], F32, tag="w1")
                ps_w2 = s3p.tile([K, 128], F32, tag="w2")
                ps_q = s3p.tile([K, K], F32, tag="q")
                ps_k = s3p.tile([K, K], F32, tag="k")
                for cc in range(CCH):
                    st, sp = (cc == 0), (cc == CCH - 1)
                    nc.tensor.matmul(ps_w1, lhsT=xp_sb[:, cc, :], rhs=wcat_sb[:, cc, 0:512], start=st, stop=sp)
                    nc.tensor.matmul(ps_w2, lhsT=xp_sb[:, cc, :], rhs=wcat_sb[:, cc, 512:640], start=st, stop=sp)
                    nc.tensor.matmul(ps_q, lhsT=wcat_sb[:, cc, 0:64], rhs=xp_sb[:, cc, :], start=st, stop=sp)
                    nc.tensor.matmul(ps_k, lhsT=wcat_sb[:, cc, 64:128], rhs=xp_sb[:, cc, :], start=st, stop=sp)
                nc.scalar.activation(out=qk_cat[:, :, 0], in_=ps_w1[:, 0:64], func=AF.Copy)
                nc.scalar.activation(out=qk_cat[:, :, 1], in_=ps_w1[:, 64:128], func=AF.Copy)
                nc.scalar.activation(out=wvg_bf[0:K, 0:384], in_=ps_w1[:, 128:512], func=AF.Copy,
                                     scale=gamma_sb[0:K, :])
                nc.scalar.activation(out=wvg_bf[0:K, 384:512], in_=ps_w2, func=AF.Copy,
                                     scale=gamma_sb[0:K, :])
                nc.scalar.activation(out=wvg_bf[K:K + 1, :], in_=bias_sb, func=AF.Copy,
                                     scale=gamma_sb[0:1, :])
                nc.scalar.activation(out=wqxp_sb, in_=ps_q, func=AF.Copy)
                nc.scalar.activation(out=wkxp_sb, in_=ps_k, func=AF.Copy)
                ps_f = s3p.tile([K, K], F32, tag="f")
                nc.tensor.matmul(ps_f, lhsT=wkxp_sb, rhs=wqxp_sb, start=True, stop=True)
                nc.scalar.activation(out=fatt_sb, in_=ps_f, func=AF.Copy)

            # ---- stage 4: QT/KT chunks; one bn_stats per chunk ------------
            with tc.tile_pool(name="s4psum", bufs=4, space="PSUM") as s4p, \
                 tc.tile_pool(name="s5psum", bufs=1, space="PSUM") as s5p, \
                 tc.tile_pool(name="s5pk", bufs=8) as s5pk:
                for ch in range(NCH):
                    ps_qk = s4p.tile([128, 128], F32, tag="qkt")
                    nc.tensor.matmul(
                        ps_qk, lhsT=ptx_sb[0:K, ch * 128:(ch + 1) * 128],
                        rhs=qk_cat[:, :, :], start=True, stop=True,
                    )
                    nc.vector.bn_stats(out=stats[:, ch, :], in_=ps_qk)

                # batched norm math: sum(x^2) = M2 + 64*mean^2 (even=q, odd=k)
                nc.vector.tensor_mul(tmpc, stats[:, :, 1], stats[:, :, 1])
                nc.vector.tensor_scalar_mul(tmpc, tmpc, 64.0)
                nc.vector.tensor_add(tmpc, tmpc, stats[:, :, 2])
                nc.scalar.activation(out=lamdq_cols, in_=tmpc, func=AF.Sqrt)
                nc.vector.reciprocal(rq_cols, lamdq_cols)
                nc.vector.tensor_mul(tmpc, stats[:, :, 4], stats[:, :, 4])
                nc.vector.tensor_scalar_mul(tmpc, tmpc, 64.0)
                nc.vector.tensor_add(tmpc, tmpc, stats[:, :, 5])
                nc.scalar.activation(out=rlk_cols, in_=tmpc, func=AF.Sqrt)
                nc.vector.reciprocal(rlk_cols, rlk_cols)

                # stage 5: A_ext = Pk^T @ [P | 1]
                ps_a = s5p.tile([K, K + 1], F32)
                for ch in range(NCH):
                    pk = s5pk.tile([128, K], BF16, tag="pk")
                    nc.vector.tensor_scalar_mul(pk, pextb_sb[:, ch, 0:K],
                                                rlk_cols[:, ch:ch + 1])
                    nc.tensor.matmul(ps_a, lhsT=pk, rhs=pextb_sb[:, ch, :],
                                     start=(ch == 0), stop=(ch == NCH - 1))
                nc.scalar.activation(out=a_s_sb, in_=ps_a, func=AF.Copy)

        # ---- stage 6: M1T/rowv, lamdv columns, rqv/irqv -------------------
        with tc.tile_pool(name="s6psum", bufs=1, space="PSUM") as s6p, \
             tc.tile_pool(name="s6pslp", bufs=2, space="PSUM") as s6lp, \
             tc.tile_pool(name="s6m", bufs=1) as s6m:
            ps_m = s6p.tile([K, K + 1], F32, tag="m")
            nc.tensor.matmul(ps_m, lhsT=fatt_sb, rhs=a_s_sb, start=True, stop=True)
            m1_sb = s6m.tile([K, K], F32, tag="m1f")
            nc.scalar.activation(out=m1_sb, in_=ps_m[:, 0:K], func=AF.Copy)
            ps_mt = s6p.tile([K, K], F32, tag="mt")
            nc.tensor.transpose(ps_mt, m1_sb, ident_sb[0:K, 0:K])
            nc.scalar.activation(out=m1sT_bf[0:K, 0:K], in_=ps_mt, func=AF.Copy)
            nc.scalar.activation(out=rowv_bf, in_=ps_m[:, K:K + 1], func=AF.Copy)
            for g in range(NCH // 16):
                ps_lp = s6lp.tile([128, 16], F32, tag="lp")
                for j in range(16):
                    ch = g * 16 + j
                    nc.tensor.matmul(ps_lp[:, j:j + 1],
                                     lhsT=ptx_sb[0:K, ch * 128:(ch + 1) * 128],
                                     rhs=rowv_bf, start=True, stop=True)
                nc.scalar.activation(out=lpre_cols[:, g * 16:(g + 1) * 16],
                                     in_=ps_lp, func=AF.Copy)
            # wvg2 = m1s_ext @ wvg  (folds M1/S/e into the V projection)
            ps_w2g = s6p.tile([K + 2, C], F32, tag="w2g")
            nc.tensor.matmul(ps_w2g, lhsT=m1sT_bf, rhs=wvg_bf, start=True, stop=True)
            nc.scalar.activation(out=wvg2_bf, in_=ps_w2g, func=AF.Copy)
            # irqv = lamdq*HW + lpre  (= lamdq*lamdv = 1/(rq*rv))
            nc.vector.tensor_scalar_mul(irqv_cols, lamdq_cols, float(HW))
            nc.vector.tensor_add(irqv_cols, irqv_cols, lpre_cols)
            # rqv = rq / (HW + lpre*rq)
            nc.vector.tensor_mul(rqv_cols, lpre_cols, rq_cols)
            nc.vector.tensor_scalar_add(rqv_cols, rqv_cols, float(HW))
            nc.vector.reciprocal(rqv_cols, rqv_cols)
            nc.vector.tensor_mul(rqv_cols, rqv_cols, rq_cols)
            # flatten lamdq/irqv to rows 64/65 of ptx: PE-transpose the cols
            # so the bounce rows land in straight n-order (n = ch*128 + p)
            with tc.tile_pool(name="s6t", bufs=1) as s6t:
                liT_bf = s6t.tile([128, 2, 128], BF16, tag="liT")
                ps_tq = s6p.tile([128, 128], F32, tag="t")
                nc.tensor.transpose(ps_tq, lamdq_cols, ident_sb)
                nc.scalar.activation(out=liT_bf[:, 0, :], in_=ps_tq, func=AF.Copy)
                ps_ti = s6p.tile([128, 128], F32, tag="t")
                nc.tensor.transpose(ps_ti, irqv_cols, ident_sb)
                nc.scalar.activation(out=liT_bf[:, 1, :], in_=ps_ti, func=AF.Copy)
                nc.gpsimd.dma_start(
                    out=flb[:, :].rearrange("o (ch p) -> ch o p", p=128), in_=liT_bf)
                nc.sync.dma_start(out=ptx_sb[K:K + 2, :], in_=flb[:, :])

        # ---- stage 7: U_ext per 512; out chunks in [HW, C] ----------------
        with tc.tile_pool(name="s7psumo", bufs=6, space="PSUM") as s7po, \
             tc.tile_pool(name="s7a", bufs=4) as s7a, \
             tc.tile_pool(name="s7o", bufs=3) as s7o:
            for g in range(NCI):
                ot = s7o.tile([128, 4, 512], BF16, tag="o")
                for j in range(4):
                    ch = 4 * g + j
                    ps_o = s7po.tile([128, 512], F32, tag="o")
                    nc.tensor.matmul(ps_o, lhsT=ptx_sb[:, ch * 128:(ch + 1) * 128],
                                     rhs=wvg2_bf, start=True, stop=True)
                    mode = ch % 8  # drain split: V-stt / ACT-scale + (G|V)-add
                    if mode in (0, 2, 4, 6):
                        nc.vector.scalar_tensor_tensor(
                            out=ot[:, j, :], in0=ps_o, scalar=rqv_cols[:, ch:ch + 1],
                            in1=xt_sb[:, ch, :], op0=ALU.mult, op1=ALU.add,
                        )
                    else:
                        at = s7a.tile([128, 512], BF16, tag="a")
                        nc.scalar.activation(out=at, in_=ps_o, func=AF.Copy,
                                             scale=rqv_cols[:, ch:ch + 1])
                        aeng = nc.vector if mode == 3 else nc.gpsimd
                        aeng.tensor_add(ot[:, j, :], at, xt_sb[:, ch, :])
                weng = nc.scalar if g % 2 == 0 else nc.sync
                weng.dma_start(out=out[:, 4 * g:4 * g + 4, :], in_=ot)

    nc.compile()
    return nc


_CACHE = {}


def _get_nc():
    if "nc" not in _CACHE:
        _CACHE["nc"] = _build()
    return _CACHE["nc"]


def _host_constants():
    if "consts" in _CACHE:
        return _CACHE["consts"]
    P = _getP()                                   # [HW, K] f32
    pext = np.ones((NCH, 128, K + 1), np.float32)
    pext[:, :, 0:K] = P.reshape(NCH, 128, K)
    pextb = np.ascontiguousarray(pext.transpose(1, 0, 2).astype(BF16_NP))  # [p,ch,K+1]
    ptbf = np.ascontiguousarray(P.T.astype(BF16_NP))              # [K, HW]
    srecol = np.zeros((K + 1, 2), np.float32)
    srecol[0:K, 0] = P.sum(axis=0, dtype=np.float64).astype(np.float32)
    srecol[K, 1] = 1.0
    srecol = np.ascontiguousarray(srecol.astype(BF16_NP))
    ident = np.eye(128, dtype=np.float32)
    _CACHE["consts"] = (pextb, ptbf, srecol, ident)
    return _CACHE["consts"]


def _make_in_map(xb, Wq, Wk, Wv, out_bias, gamma):
    pextb, ptbf, srecol, ident = _host_constants()
    wcat_full = np.concatenate([Wq.T, Wk.T, Wv.T], axis=1)        # [C, 640]
    wcat = np.ascontiguousarray(
        wcat_full.reshape(CCH, 128, 640).transpose(1, 0, 2).astype(BF16_NP))
    biasg = np.ascontiguousarray(out_bias.reshape(1, C))
    gam = gamma.reshape(1, 1)
    # x^T pre-permuted to the SBUF-resident layout: [p, h, c], n = h*128 + p
    xtr = np.ascontiguousarray(
        xb.T.reshape(NCH, 128, C).transpose(1, 0, 2)).astype(BF16_NP)
    return {
        "xtr": xtr,
        "pextb": pextb, "ptbf": ptbf, "wcat": wcat,
        "ident": ident, "biasg": biasg, "gam": gam, "srecol": srecol,
    }


def kernel(x, Wq, Wk, Wv, out_bias, gamma):
    x = np.asarray(x, dtype=np.float32)
    Wq = np.asarray(Wq, dtype=np.float32)
    Wk = np.asarray(Wk, dtype=np.float32)
    Wv = np.asarray(Wv, dtype=np.float32)
    out_bias = np.asarray(out_bias, dtype=np.float32)
    gamma = np.asarray(gamma, dtype=np.float32)

    x2 = x.reshape(B, C, HW)
    in_maps = [_make_in_map(x2[b], Wq, Wk, Wv, out_bias, gamma) for b in range(B)]

    nc = _get_nc()
    res = run_bass_kernel_spmd(nc, in_maps, core_ids=list(range(B)))
    # device out is [p, ch, c] with n = ch*128 + p -> [c, ch, p] = [C, HW]
    out = np.stack(
        [res.results[b]["out"].astype(np.float32).transpose(2, 1, 0)
         for b in range(B)], axis=0)
    return np.ascontiguousarray(out.reshape(B, C, H, W))


def bench(inputs, core_id=0):
    """Single-core traced run for timing (same SPMD program on every core)."""
    res = bench_res(inputs, core_id)
    return res.exec_time_ns


def bench_res(inputs, core_id=0):
    x = np.asarray(inputs["x"], dtype=np.float32)
    xb = x.reshape(B, C, HW)[0]
    in_map = _make_in_map(
        xb,
        np.asarray(inputs["Wq"], dtype=np.float32),
        np.asarray(inputs["Wk"], dtype=np.float32),
        np.asarray(inputs["Wv"], dtype=np.float32),
        np.asarray(inputs["out_bias"], dtype=np.float32),
        np.asarray(inputs["gamma"], dtype=np.float32),
    )
    nc = _get_nc()
    return run_bass_kernel_spmd(nc, [in_map], core_ids=[core_id], trace=True)


if __name__ == "__main__":
    rng = np.random.default_rng(0)
    x = rng.standard_normal((B, C, H, W), dtype=np.float32)
    Wq = (rng.standard_normal((K, C)) * 0.05).astype(np.float32)
    Wk = (rng.standard_normal((K, C)) * 0.05).astype(np.float32)
    Wv = (rng.standard_normal((C, C)) * 0.05).astype(np.float32)
    ob = (rng.standard_normal((1, C, 1)) * 0.01).astype(np.float32)
    g = (rng.standard_normal((1,)) * 0.5).astype(np.float32)
    y = kernel(x=x, Wq=Wq, Wk=Wk, Wv=Wv, out_bias=ob, gamma=g)
    print("out", y.shape, y.dtype, float(np.abs(y).mean()))


# revision 38
# speedup vs baseline: 1.1185x; 1.1185x over previous
"""DCT non-local attention (nn_DCTNLAttention11) Trainium2 kernel.

Data-parallel over batch B=8 across 8 NeuronCores; each core processes one
batch element [C=512, HW=16384].  All constants derived from the DCT basis P
are precomputed on host; the per-core device program is:

  1. xT [HW, C] bf16 is DMAed ONCE into a resident SBUF tile
     [128p, 128h, 512c] (n = h*128+p); xPT = P^T @ x^T accumulates 128
     matmuls against the resident chunks as the DMA slices land.
  2. xP (PE transposes), then W-projections off xP (tiny matmuls):
     WqxP^T/WkxP^T/WvxP^T (q/k column-interleaved), WqxP/WkxP, fatt.
  3. Per-n norms: QT/KT chunks [128,128] (q/k interleaved columns) via
     PT-chunk-stationary matmuls; ONE bn_stats per chunk reads PSUM and its
     even/odd stats give sum(q^2) and sum(k^2); batched column math.
  4. Pk = P * (1/lamdk) (ACT, bf16); A_ext = Pk^T @ [P|1] accumulated, bf16.
  5. M1T/rowv = fatt^T @ [A|s]; lamdv_pre columns (batched [128,16] psums);
     lamdq and irqv = lamdq*HW + lamdv_pre (= 1/(rq*rv)) flattened to rows
     64/65 of the PT tensor via a DRAM bounce.
  6. Per 512-col group: U_ext [65,512] = [M1;S;e]^T @ [PT; lamdq; irqv]
     (rows 0-63 = M1^T PT + S (x) lamdq, row 64 = irqv); ACT-drain to bf16
     t (row 64 scales the bias so the rqv drain scale cancels there).
     Per 128-n chunk: psum [128n, 512c] = t_chunk^T @ [gamma*WvxP^T;
     gamma*bias]; fused drain out = psum*rqv[n] + xT_chunk (bf16), split
     Vector/GpSimd; out written bf16 in [HW, C]; host transposes + casts.
"""

import numpy as np
import ml_dtypes
from contextlib import ExitStack

import concourse.bass as bass
import concourse.bacc as bacc
import concourse.tile as tile
from concourse import mybir
from concourse.bass_utils import run_bass_kernel_spmd

F32 = mybir.dt.float32
BF16 = mybir.dt.bfloat16
AF = mybir.ActivationFunctionType
ALU = mybir.AluOpType
BF16_NP = ml_dtypes.bfloat16

B, C, H, W = 8, 512, 128, 128
HW = H * W          # 16384
K = 64              # kept DCT coefficients (8x8 band)
NCH = HW // 128     # 128 n-chunks of 128
NCI = HW // 512     # 32 n-chunks of 512
CCH = C // 128      # 4 c-chunks


def _getP():
    """DCT projection matrix P [HW, K], faithful to the reference."""
    Hs, Ws = H, W
    k = (0, 8, 0, 8)
    ind_h = 2.0 * np.arange(Hs) + 1.0
    Dht = np.stack(
        [np.sqrt(2.0) / np.sqrt(Hs) * np.cos(u * ind_h * np.pi / (2.0 * Hs)) for u in range(Hs)]
    ).astype(np.float32)
    Dht[0, :] = 1.0 / np.sqrt(Hs)
    Dh = Dht.T[:, k[0]:k[1]]
    ind_w = 2.0 * np.arange(Ws) + 1.0
    Dvt = np.stack(
        [np.sqrt(2.0) / np.sqrt(Hs) * np.cos(u * ind_w * np.pi / (2.0 * Ws)) for u in range(Ws)]
    ).astype(np.float32)
    Dvt[0, :] = 1.0 / np.sqrt(Ws)
    Dv = Dvt.T[:, k[2]:k[3]]
    P = np.einsum("hu,wv->hwuv", Dh, Dv).reshape(Hs * Ws, (k[1] - k[0]) * (k[3] - k[2]))
    return np.ascontiguousarray(P.astype(np.float32))


def _build():
    nc = bacc.Bacc("TRN2", target_bir_lowering=False, debug=False, enable_asserts=False)

    xtr = nc.dram_tensor("xtr", [128, NCH, C], BF16, kind="ExternalInput")
    pextb = nc.dram_tensor("pextb", [128, NCH, K + 1], BF16, kind="ExternalInput")
    ptbf = nc.dram_tensor("ptbf", [K, HW], BF16, kind="ExternalInput")
    wcat = nc.dram_tensor("wcat", [128, CCH, 640], BF16, kind="ExternalInput")
    ident = nc.dram_tensor("ident", [128, 128], F32, kind="ExternalInput")
    biasg = nc.dram_tensor("biasg", [1, C], F32, kind="ExternalInput")
    gam = nc.dram_tensor("gam", [1, 1], F32, kind="ExternalInput")
    srecol = nc.dram_tensor("srecol", [K + 1, 2], BF16, kind="ExternalInput")
    out = nc.dram_tensor("out", [128, NCH, C], BF16, kind="ExternalOutput")
    flb = nc.dram_tensor("flbounce", [2, HW], BF16, kind="Internal")

    with tile.TileContext(nc) as tc, ExitStack() as top:
        consts = top.enter_context(tc.tile_pool(name="consts", bufs=1))

        # resident x^T: n = h*128 + p on (p, h); 512 c contiguous
        xt_sb = consts.tile([128, NCH, C], BF16)
        # [PT ; lamdq-row ; irqv-row]
        ptx_sb = consts.tile([K + 2, HW], BF16)
        ident_sb = consts.tile([128, 128], F32)
        bias_sb = consts.tile([1, C], F32)
        gamma_sb = consts.tile([128, 1], F32)
        xpt_sb = consts.tile([K, C], F32)            # xP^T
        xp_sb = consts.tile([128, CCH, K], BF16)     # xP chunks (c on partitions)
        qk_cat = consts.tile([K, 64, 2], BF16)       # q/k interleaved columns
        wqxp_sb = consts.tile([K, K], F32)
        wkxp_sb = consts.tile([K, K], F32)
        fatt_sb = consts.tile([K, K], F32)
        a_s_sb = consts.tile([K, K + 1], F32)        # [A | s]
        m1sT_bf = consts.tile([K + 1, K + 2], BF16)  # [M1 | srow-col | e-col]
        rowv_bf = consts.tile([K, 1], BF16)
        wvg_bf = consts.tile([K + 1, C], BF16)       # [gamma*WvxP^T ; gamma*bias]
        wvg2_bf = consts.tile([K + 2, C], BF16)      # [m1s_ext @ wvg]
        stats = consts.tile([128, NCH, 6], F32)      # bn_stats per chunk
        tmpc = consts.tile([128, NCH], F32)
        lamdq_cols = consts.tile([128, NCH], F32)
        rq_cols = consts.tile([128, NCH], F32)
        rlk_cols = consts.tile([128, NCH], F32)
        lpre_cols = consts.tile([128, NCH], F32)
        rqv_cols = consts.tile([128, NCH], F32)
        irqv_cols = consts.tile([128, NCH], F32)

        # ---- stage 1: xPT = P^T @ x^T  ------------------------------------
        # P chunks (from pextb) land first on the scalar ring; x slices
        # stream on sync+gpsimd rings into the resident tile; consts follow.
        with tc.tile_pool(name="pextp", bufs=1) as pextp:
            pextb_sb = pextp.tile([128, NCH, K + 1], BF16)
            with tc.tile_pool(name="s1psum", bufs=1, space="PSUM") as s1p:
                nc.scalar.dma_start(out=pextb_sb[:, 0:16, :], in_=pextb[:, 0:16, :])
                nc.scalar.dma_start(out=pextb_sb[:, 16:NCH, :], in_=pextb[:, 16:NCH, :])
                # x slices: small first for a fast pipeline start, then 4-chunk
                # blocks (4KB contiguous per partition) over four trigger rings
                bounds = [0, 2, 4]
                while bounds[-1] < NCH:
                    bounds.append(min(bounds[-1] + 4, NCH))
                rings = [nc.sync, nc.gpsimd]
                for s in range(len(bounds) - 1):
                    lo, hi = bounds[s], bounds[s + 1]
                    eng = rings[s % 2]
                    eng.dma_start(out=xt_sb[:, lo:hi, :], in_=xtr[:, lo:hi, :])

                ps_xpt = s1p.tile([K, C], F32)
                for h in range(NCH):
                    nc.tensor.matmul(
                        ps_xpt, lhsT=pextb_sb[:, h, 0:K], rhs=xt_sb[:, h, :],
                        start=(h == 0), stop=(h == NCH - 1),
                    )
                # defer const loads behind the stage-1 stream
                nc.scalar.dma_start(out=ptx_sb[0:K, :], in_=ptbf[:, :])
                nc.scalar.dma_start(out=ident_sb, in_=ident[:, :])
                nc.scalar.dma_start(out=bias_sb, in_=biasg[:, :])
                nc.vector.memset(m1sT_bf, 0.0)
                nc.scalar.dma_start(out=m1sT_bf[:, K:K + 2], in_=srecol[:, :])
                nc.gpsimd.dma_start(out=gamma_sb, in_=gam[:, :].to_broadcast((128, 1)))
                nc.scalar.activation(out=xpt_sb, in_=ps_xpt, func=AF.Copy)

            # ---- stage 2+3: xP via PE transpose; W projections ------------
            with tc.tile_pool(name="wcatp", bufs=1) as wcatp, \
                 tc.tile_pool(name="s2psum", bufs=2, space="PSUM") as s2p, \
                 tc.tile_pool(name="s3psum", bufs=1, space="PSUM") as s3p:
                wcat_sb = wcatp.tile([128, CCH, 640], BF16)
                nc.scalar.dma_start(out=wcat_sb, in_=wcat[:, :, :])
                for cc in range(CCH):
                    ps_tr = s2p.tile([128, K], F32, tag="tr")
                    nc.tensor.transpose(
                        ps_tr, xpt_sb[:, cc * 128:(cc + 1) * 128], ident_sb[0:K, 0:K]
                    )
                    nc.scalar.activation(out=xp_sb[:, cc, :], in_=ps_tr, func=AF.Copy)

                ps_w1 = s3p.tile([K, 512], F32, tag="w1")
                ps_w2 = s3p.tile([K, 128], F32, tag="w2")
                ps_q = s3p.tile([K, K], F32, tag="q")
                ps_k = s3p.tile([K, K], F32, tag="k")
                for cc in range(CCH):
                    st, sp = (cc == 0), (cc == CCH - 1)
                    nc.tensor.matmul(ps_w1, lhsT=xp_sb[:, cc, :], rhs=wcat_sb[:, cc, 0:512], start=st, stop=sp)
                    nc.tensor.matmul(ps_w2, lhsT=xp_sb[:, cc, :], rhs=wcat_sb[:, cc, 512:640], start=st, stop=sp)
                    nc.tensor.matmul(ps_q, lhsT=wcat_sb[:, cc, 0:64], rhs=xp_sb[:, cc, :], start=st, stop=sp)
                    nc.tensor.matmul(ps_k, lhsT=wcat_sb[:, cc, 64:128], rhs=xp_sb[:, cc, :], start=st, stop=sp)
                nc.scalar.activation(out=qk_cat[:, :, 0], in_=ps_w1[:, 0:64], func=AF.Copy)
                nc.scalar.activation(out=qk_cat[:, :, 1], in_=ps_w1[:, 64:128], func=AF.Copy)
                nc.scalar.activation(out=wvg_bf[0:K, 0:384], in_=ps_w1[:, 128:512], func=AF.Copy,
                                     scale=gamma_sb[0:K, :])
                nc.scalar.activation(out=wvg_bf[0:K, 384:512], in_=ps_w2, func=AF.Copy,
                                     scale=gamma_sb[0:K, :])
                nc.scalar.activation(out=wvg_bf[K:K + 1, :], in_=bias_sb, func=AF.Copy,
                                     scale=gamma_sb[0:1, :])
                nc.scalar.activation(out=wqxp_sb, in_=ps_q, func=AF.Copy)
                nc.scalar.activation(out=wkxp_sb, in_=ps_k, func=AF.Copy)
                ps_f = s3p.tile([K, K], F32, tag="f")
                nc.tensor.matmul(ps_f, lhsT=wkxp_sb, rhs=wqxp_sb, start=True, stop=True)
                nc.scalar.activation(out=fatt_sb, in_=ps_f, func=AF.Copy)

            # ---- stage 4: QT/KT chunks; one bn_stats per chunk ------------
            with tc.tile_pool(name="s4psum", bufs=4, space="PSUM") as s4p, \
                 tc.tile_pool(name="s5psum", bufs=1, space="PSUM") as s5p, \
                 tc.tile_pool(name="s5pk", bufs=8) as s5pk:
                for ch in range(NCH):
                    ps_qk = s4p.tile([128, 128], F32, tag="qkt")
                    nc.tensor.matmul(
                        ps_qk, lhsT=ptx_sb[0:K, ch * 128:(ch + 1) * 128],
                        rhs=qk_cat[:, :, :], start=True, stop=True,
                    )
                    nc.vector.bn_stats(out=stats[:, ch, :], in_=ps_qk)

                # batched norm math: sum(x^2) = M2 + 64*mean^2 (even=q, odd=k)
                nc.vector.tensor_mul(tmpc, stats[:, :, 1], stats[:, :, 1])
                nc.vector.tensor_scalar_mul(tmpc, tmpc, 64.0)
                nc.vector.tensor_add(tmpc, tmpc, stats[:, :, 2])
                nc.scalar.activation(out=lamdq_cols, in_=tmpc, func=AF.Sqrt)
                nc.vector.reciprocal(rq_cols, lamdq_cols)
                nc.vector.tensor_mul(tmpc, stats[:, :, 4], stats[:, :, 4])
                nc.vector.tensor_scalar_mul(tmpc, tmpc, 64.0)
                nc.vector.tensor_add(tmpc, tmpc, stats[:, :, 5])
                nc.scalar.activation(out=rlk_cols, in_=tmpc, func=AF.Sqrt)
                nc.vector.reciprocal(rlk_cols, rlk_cols)

                # stage 5: A_ext = Pk^T @ [P | 1]
                ps_a = s5p.tile([K, K + 1], F32)
                for ch in range(NCH):
                    pk = s5pk.tile([128, K], BF16, tag="pk")
                    nc.vector.tensor_scalar_mul(pk, pextb_sb[:, ch, 0:K],
                                                rlk_cols[:, ch:ch + 1])
                    nc.tensor.matmul(ps_a, lhsT=pk, rhs=pextb_sb[:, ch, :],
                                     start=(ch == 0), stop=(ch == NCH - 1))
                nc.scalar.activation(out=a_s_sb, in_=ps_a, func=AF.Copy)

        # ---- stage 6: M1T/rowv, lamdv columns, rqv/irqv -------------------
        with tc.tile_pool(name="s6psum", bufs=1, space="PSUM") as s6p, \
             tc.tile_pool(name="s6pslp", bufs=2, space="PSUM") as s6lp, \
             tc.tile_pool(name="s6m", bufs=1) as s6m:
            ps_m = s6p.tile([K, K + 1], F32, tag="m")
            nc.tensor.matmul(ps_m, lhsT=fatt_sb, rhs=a_s_sb, start=True, stop=True)
            m1_sb = s6m.tile([K, K], F32, tag="m1f")
            nc.scalar.activation(out=m1_sb, in_=ps_m[:, 0:K], func=AF.Copy)
            ps_mt = s6p.tile([K, K], F32, tag="mt")
            nc.tensor.transpose(ps_mt, m1_sb, ident_sb[0:K, 0:K])
            nc.scalar.activation(out=m1sT_bf[0:K, 0:K], in_=ps_mt, func=AF.Copy)
            nc.scalar.activation(out=rowv_bf, in_=ps_m[:, K:K + 1], func=AF.Copy)
            for g in range(NCH // 16):
                ps_lp = s6lp.tile([128, 16], F32, tag="lp")
                for j in range(16):
                    ch = g * 16 + j
                    nc.tensor.matmul(ps_lp[:, j:j + 1],
                                     lhsT=ptx_sb[0:K, ch * 128:(ch + 1) * 128],
                                     rhs=rowv_bf, start=True, stop=True)
                nc.scalar.activation(out=lpre_cols[:, g * 16:(g + 1) * 16],
                                     in_=ps_lp, func=AF.Copy)
            # wvg2 = m1s_ext @ wvg  (folds M1/S/e into the V projection)
            ps_w2g = s6p.tile([K + 2, C], F32, tag="w2g")
            nc.tensor.matmul(ps_w2g, lhsT=m1sT_bf, rhs=wvg_bf, start=True, stop=True)
            nc.scalar.activation(out=wvg2_bf, in_=ps_w2g, func=AF.Copy)
            # irqv = lamdq*HW + lpre  (= lamdq*lamdv = 1/(rq*rv))
            nc.vector.tensor_scalar_mul(irqv_cols, lamdq_cols, float(HW))
            nc.vector.tensor_add(irqv_cols, irqv_cols, lpre_cols)
            # rqv = rq / (HW + lpre*rq)
            nc.vector.tensor_mul(rqv_cols, lpre_cols, rq_cols)
            nc.vector.tensor_scalar_add(rqv_cols, rqv_cols, float(HW))
            nc.vector.reciprocal(rqv_cols, rqv_cols)
            nc.vector.tensor_mul(rqv_cols, rqv_cols, rq_cols)
            # flatten lamdq/irqv to rows 64/65 of ptx: PE-transpose the cols
            # so the bounce rows land in straight n-order (n = ch*128 + p)
            with tc.tile_pool(name="s6t", bufs=1) as s6t:
                liT_bf = s6t.tile([128, 2, 128], BF16, tag="liT")
                ps_tq = s6p.tile([128, 128], F32, tag="t")
                nc.tensor.transpose(ps_tq, lamdq_cols, ident_sb)
                nc.scalar.activation(out=liT_bf[:, 0, :], in_=ps_tq, func=AF.Copy)
                ps_ti = s6p.tile([128, 128], F32, tag="t")
                nc.tensor.transpose(ps_ti, irqv_cols, ident_sb)
                nc.scalar.activation(out=liT_bf[:, 1, :], in_=ps_ti, func=AF.Copy)
                nc.gpsimd.dma_start(
                    out=flb[:, :].rearrange("o (ch p) -> ch o p", p=128), in_=liT_bf)
                nc.sync.dma_start(out=ptx_sb[K:K + 2, :], in_=flb[:, :])

        # ---- stage 7: U_ext per 512; out chunks in [HW, C] ----------------
        with tc.tile_pool(name="s7psumo", bufs=6, space="PSUM") as s7po, \
             tc.tile_pool(name="s7a", bufs=4) as s7a, \
             tc.tile_pool(name="s7o", bufs=3) as s7o:
            for g in range(NCI):
                ot = s7o.tile([128, 4, 512], BF16, tag="o")
                for j in range(4):
                    ch = 4 * g + j
                    ps_o = s7po.tile([128, 512], F32, tag="o")
                    nc.tensor.matmul(ps_o, lhsT=ptx_sb[:, ch * 128:(ch + 1) * 128],
                                     rhs=wvg2_bf, start=True, stop=True)
                    mode = ch % 8  # drain split: V-stt / ACT-scale + (G|V)-add
                    if mode in (0, 2, 4, 6):
                        nc.vector.scalar_tensor_tensor(
                            out=ot[:, j, :], in0=ps_o, scalar=rqv_cols[:, ch:ch + 1],
                            in1=xt_sb[:, ch, :], op0=ALU.mult, op1=ALU.add,
                        )
                    else:
                        at = s7a.tile([128, 512], BF16, tag="a")
                        nc.scalar.activation(out=at, in_=ps_o, func=AF.Copy,
                                             scale=rqv_cols[:, ch:ch + 1])
                        aeng = nc.vector if mode == 3 else nc.gpsimd
                        aeng.tensor_add(ot[:, j, :], at, xt_sb[:, ch, :])
                weng = nc.scalar if g % 2 == 0 else nc.sync
                weng.dma_start(out=out[:, 4 * g:4 * g + 4, :], in_=ot)

    nc.compile()
    return nc


_CACHE = {}


def _get_nc():
    if "nc" not in _CACHE:
        _CACHE["nc"] = _build()
    return _CACHE["nc"]


def _host_constants():
    if "consts" in _CACHE:
        return _CACHE["consts"]
    P = _getP()                                   # [HW, K] f32
    pext = np.ones((NCH, 128, K + 1), np.float32)
    pext[:, :, 0:K] = P.reshape(NCH, 128, K)
    pextb = np.ascontiguousarray(pext.transpose(1, 0, 2).astype(BF16_NP))  # [p,ch,K+1]
    ptbf = np.ascontiguousarray(P.T.astype(BF16_NP))              # [K, HW]
    srecol = np.zeros((K + 1, 2), np.float32)
    srecol[0:K, 0] = P.sum(axis=0, dtype=np.float64).astype(np.float32)
    srecol[K, 1] = 1.0
    srecol = np.ascontiguousarray(srecol.astype(BF16_NP))
    ident = np.eye(128, dtype=np.float32)
    _CACHE["consts"] = (pextb, ptbf, srecol, ident)
    return _CACHE["consts"]


def _make_in_map(xb, Wq, Wk, Wv, out_bias, gamma):
    pextb, ptbf, srecol, ident = _host_constants()
    wcat_full = np.concatenate([Wq.T, Wk.T, Wv.T], axis=1)        # [C, 640]
    wcat = np.ascontiguousarray(
        wcat_full.reshape(CCH, 128, 640).transpose(1, 0, 2).astype(BF16_NP))
    biasg = np.ascontiguousarray(out_bias.reshape(1, C))
    gam = gamma.reshape(1, 1)
    # x^T pre-permuted to the SBUF-resident layout: [p, h, c], n = h*128 + p
    xtr = np.ascontiguousarray(
        xb.T.reshape(NCH, 128, C).transpose(1, 0, 2)).astype(BF16_NP)
    return {
        "xtr": xtr,
        "pextb": pextb, "ptbf": ptbf, "wcat": wcat,
        "ident": ident, "biasg": biasg, "gam": gam, "srecol": srecol,
    }


def kernel(x, Wq, Wk, Wv, out_bias, gamma):
    x = np.asarray(x, dtype=np.float32)
    Wq = np.asarray(Wq, dtype=np.float32)
    Wk = np.asarray(Wk, dtype=np.float32)
    Wv = np.asarray(Wv, dtype=np.float32)
    out_bias = np.asarray(out_bias, dtype=np.float32)
    gamma = np.asarray(gamma, dtype=np.float32)

    x2 = x.reshape(B, C, HW)
    in_maps = [_make_in_map(x2[b], Wq, Wk, Wv, out_bias, gamma) for b in range(B)]

    nc = _get_nc()
    res = run_bass_kernel_spmd(nc, in_maps, core_ids=list(range(B)))
    # device out is [p, ch, c] with n = ch*128 + p -> [c, ch, p] = [C, HW]
    out = np.stack(
        [res.results[b]["out"].astype(np.float32).transpose(2, 1, 0)
         for b in range(B)], axis=0)
    return np.ascontiguousarray(out.reshape(B, C, H, W))


def bench(inputs, core_id=0):
    """Single-core traced run for timing (same SPMD program on every core)."""
    res = bench_res(inputs, core_id)
    return res.exec_time_ns


def bench_res(inputs, core_id=0):
    x = np.asarray(inputs["x"], dtype=np.float32)
    xb = x.reshape(B, C, HW)[0]
    in_map = _make_in_map(
        xb,
        np.asarray(inputs["Wq"], dtype=np.float32),
        np.asarray(inputs["Wk"], dtype=np.float32),
        np.asarray(inputs["Wv"], dtype=np.float32),
        np.asarray(inputs["out_bias"], dtype=np.float32),
        np.asarray(inputs["gamma"], dtype=np.float32),
    )
    nc = _get_nc()
    return run_bass_kernel_spmd(nc, [in_map], core_ids=[core_id], trace=True)


if __name__ == "__main__":
    rng = np.random.default_rng(0)
    x = rng.standard_normal((B, C, H, W), dtype=np.float32)
    Wq = (rng.standard_normal((K, C)) * 0.05).astype(np.float32)
    Wk = (rng.standard_normal((K, C)) * 0.05).astype(np.float32)
    Wv = (rng.standard_normal((C, C)) * 0.05).astype(np.float32)
    ob = (rng.standard_normal((1, C, 1)) * 0.01).astype(np.float32)
    g = (rng.standard_normal((1,)) * 0.5).astype(np.float32)
    y = kernel(x=x, Wq=Wq, Wk=Wk, Wv=Wv, out_bias=ob, gamma=g)
    print("out", y.shape, y.dtype, float(np.abs(y).mean()))
